# revision 18
# baseline (speedup 1.0000x reference)
"""Causal attention with L2-normalized Q/K — Trainium2 Bass kernel.

Problem shapes (hardcoded): X [2, 2048, 1024], Wq/Wk/Wv [1024, 1024],
Wo [1024, 1024], bo [1024]; H=16 heads, d_head=64.

Sharding: 8 cores = 2 batches x 4 head-groups (4 heads each).
Core c handles batch b=c//4, heads 4*(c%4)..4*(c%4)+3.
Each core computes QKV projections for its head slice, per-head
normalized causal attention, and a partial output projection
V_hat @ Wo[slice]. The partials are summed with per-q-chunk bf16
ReduceScatters across the 4 cores of the batch; the host reassembles
the row strips.

v3 design notes (vs the v2 baseline at ~277us):
- Q is stored in the SAME natural pair layout as K (head 2hp at
  partitions 0:64, head 2hp+1 at 64:128). Score matmuls contract over
  64 partitions with matching base offsets (PE cost is per moving
  column, independent of contraction depth), so no zero-padding DMAs
  and half-size score LDWEIGHTS. This also fixes v2's head pairing:
  v2 overwrote its 2 Q slots per head-pair so all scores used the
  last pair's Q (numerically masked by the near-uniform attention of
  L2-normalized Q/K, but ~9e-3 of avoidable error).
- Score pairs land in one 2-bank PSUM tile [128, 2, 512] so ONE ACT
  exp covers both heads of a pair per k-tile (ACT per-instruction
  overhead ~300ns; halving instruction count saves ~45us of ACT).
  Diagonal mask multiply also covers the pair in one DVE op.
- Softmax denominators: ots are evacuated to SBUF immediately after
  the AV accumulation stops (frees PSUM banks fast), the two den rows
  get a native-DVE reciprocal (no ACT Ln/Exp chain, no table traffic),
  and one f32r selector matmul broadcasts both reciprocal rows across
  partitions.
- Input loads are split per (i-tile, chunk) and spread across the two
  HWDGE queues (sync + scalar): proj(0) needs only wq + chunk-0 xt
  (1.5MB) instead of the whole 5.5MB preload, moving first-matmul
  from ~18us to ~6us.
- PSUM: score-pair pool 2x[128,2,512] (4 banks) + 2 AV accumulators
  + 2 general banks (projection/su/rb/rbo/yp) = 8 banks exactly.
- Output: per q-chunk, per 512-col half, fp16 ReduceScatter of the
  [4,128,512] partials; the last chunk's halves are split into
  quarter RS ops so the tail exposure is one 0.25MB collective.
"""

import math
import numpy as np
from contextlib import ExitStack

import concourse.bass as bass
import concourse.tile as tile
from concourse import mybir
from concourse.bass import _add_dep_helper as add_dep
from concourse.bass_utils import run_bass_kernel_spmd

F32 = mybir.dt.float32
F32R = mybir.dt.float32r
F16 = mybir.dt.float16
AF = mybir.ActivationFunctionType

B, N, D, H, DH = 2, 2048, 1024, 16, 64
NH = 4            # heads per core
J = NH * DH       # head dims per core = 256
P = 128
NQ = 512          # q chunk (moving free dim / psum bank)
NKT = N // P      # 16 k-tiles per head
ID = D // P       # 8 i-tiles of d_model
VW = DH + 1       # 65: V columns + ones column
NQC = N // NQ     # 4 q-chunks

_MAX_WAITS = 1


def _split_excess_waits(nc, limit=_MAX_WAITS):
    """This walrus build allows very few sem waits per instruction.
    Tile can emit many (kernel-tail Drain, collectives reading
    many-writer DRAM). Move excess waits onto injected same-engine
    NoOps right before the instruction; in-order execution preserves
    the semantics."""
    ctr = 0
    for fn in nc.m.functions:
        for bb in fn.blocks:
            out = []
            changed = False
            for ins in bb.instructions:
                si = ins.sync_info
                waits = list(si.on_wait) if si and si.on_wait else []
                if len(waits) > limit:
                    changed = True
                    chunks = [
                        waits[i : i + limit] for i in range(0, len(waits), limit)
                    ]
                    for ch in chunks[:-1]:
                        nop = mybir.InstNoOp(
                            name=f"I-waitsplit-{ctr}", ins=[], outs=[]
                        )
                        ctr += 1
                        nop.engine = ins.engine
                        nop.sync_info = mybir.SyncInfo(on_wait=ch, on_update=[])
                        out.append(nop)
                    ins.sync_info = mybir.SyncInfo(
                        on_wait=chunks[-1], on_update=list(si.on_update or [])
                    )
                out.append(ins)
            if changed:
                bb.instructions = out


def _build():
    nc = bass.Bass("TRN2", target_bir_lowering=False, debug=False, num_devices=8)

    xt = nc.dram_tensor("xt", [ID, NQC, P, NQ], F16, kind="ExternalInput").ap()
    wq = nc.dram_tensor("wq", [2, P, 4, J], F16, kind="ExternalInput").ap()
    wk = nc.dram_tensor("wk", [2, P, 4, J], F16, kind="ExternalInput").ap()
    wv = nc.dram_tensor("wv", [2, P, 4, J], F16, kind="ExternalInput").ap()
    wo = nc.dram_tensor("wo", [P, 2, D], F16, kind="ExternalInput").ap()
    bias4 = nc.dram_tensor("bias4", [D], F32, kind="ExternalInput").ap()
    mask2d = nc.dram_tensor("mask2d", [P, 2, P], F16, kind="ExternalInput").ap()
    ones2d = nc.dram_tensor("ones2d", [P, 2], F16, kind="ExternalInput").ap()
    sel2d = nc.dram_tensor("sel2d", [2, P], F32R, kind="ExternalInput").ap()
    selpd = nc.dram_tensor("selpd", [VW, 2, P], F32R, kind="ExternalInput").ap()
    # output: per q-chunk, 2 column halves of this core's 128-row strip
    y_ext = nc.dram_tensor(
        "y", [NQC, 2, P, NQ], F16, kind="ExternalOutput"
    ).ap()

    yparts = [
        [nc.dram_tensor(f"ypart{qc}_{mc}", [4, P, NQ], F16) for mc in range(2)]
        for qc in range(NQC - 1)
    ]
    yrss = [
        [nc.dram_tensor(f"yrs{qc}_{mc}", [P, NQ], F16) for mc in range(2)]
        for qc in range(NQC - 1)
    ]
    # last chunk: one 1MB RS (a single ~20us window beats two serial
    # ~13us ones on the exposed tail)
    ypart3 = nc.dram_tensor("ypart3", [4, P, D], F16)
    yrs3 = nc.dram_tensor("yrs3", [P, D], F16)

    with tile.TileContext(nc) as tc:
        with ExitStack() as ctx:
            sb = ctx.enter_context(tc.tile_pool(name="sb", bufs=1))
            ps = ctx.enter_context(tc.tile_pool(name="ps", bufs=1, space="PSUM"))

            # ---- loads: split per (i, c) and spread across the two
            # HWDGE queues (sync + scalar). gpsimd SW-DGE loads hang
            # the device; vector/tensor cannot issue DMA on TRN2. ----
            def load_w_half(eng, ap_in, nm, h):
                t = sb.tile([P, 4, J], F16, tag=f"{nm}{h}", name=f"{nm}{h}")
                eng.dma_start(t[:], ap_in[h])
                return t

            # critical path: wq + half of chunk-0 x on sync, the other
            # half of chunk-0 x then wk/wv on scalar
            wq_h = [load_w_half(nc.sync, wq, "wq", h) for h in range(2)]

            xt_sb = [[None] * NQC for _ in range(ID)]

            def load_x(eng, i, c):
                t = sb.tile([P, NQ], F16, tag=f"x{i}_{c}", name=f"x{i}_{c}")
                eng.dma_start(t[:], xt[i, c])
                xt_sb[i][c] = t

            def load_x_chunk(eng, c):
                for i in range(ID):
                    load_x(eng, i, c)

            for i in range(4):
                load_x(nc.scalar, i, 0)
            for i in range(4, ID):
                load_x(nc.sync, i, 0)
            wk_h = [load_w_half(nc.scalar, wk, "wk", h) for h in range(2)]
            wv_h = [load_w_half(nc.scalar, wv, "wv", h) for h in range(2)]
            load_x_chunk(nc.sync, 1)

            # small constants on sync, ordered by first use (scalar
            # engine carries only wk/wv: its DMA instructions execute
            # ahead of all its ACT work, so anything queued there
            # delays every norm chain)
            mask2_sb = sb.tile([P, 2, P], F16, tag="mask2")
            nc.sync.dma_start(mask2_sb[:], mask2d)
            ones2_sb = sb.tile([P, 2], F16, tag="ones2")
            nc.sync.dma_start(ones2_sb[:], ones2d)
            sel2_sb = sb.tile([2, P], F32R, tag="sel2")
            nc.sync.dma_start(sel2_sb[:], sel2d)
            selp_sb = sb.tile([VW, 2, P], F32R, tag="selp")
            nc.sync.dma_start(selp_sb[:], selpd)

            # ---- static SBUF state (per q-chunk tiles; Q uses the same
            # natural pair layout as K: head 2hp at partitions 0:64,
            # head 2hp+1 at 64:128 of slot hp) ----
            qt_c = [
                sb.tile([P, 2, NQ], F16, tag=f"qtc{c}", name=f"qtc{c}")
                for c in range(NQC)
            ]
            kt_c = [
                sb.tile([P, 2, NQ], F16, tag=f"ktc{c}", name=f"ktc{c}")
                for c in range(NQC)
            ]
            v_c = [
                sb.tile([P, 4, NH, VW], F16, tag=f"vc{c}", name=f"vc{c}")
                for c in range(NQC)
            ]
            # set the V ones column via gpsimd memset (a broadcast DMA
            # here costs ~20us: 2048 two-byte descriptors)
            for c in range(NQC):
                nc.gpsimd.memset(v_c[c][:, :, :, DH : DH + 1], 1.0)
            wo_sb = sb.tile([P, 2, D], F16, tag="wo")
            nc.sync.dma_start(wo_sb[:], wo)
            bias_sb = sb.tile([P, D], F32, tag="bias")
            nc.sync.dma_start(
                bias_sb[:],
                bias4.rearrange("(a m) -> a m", a=1).to_broadcast((P, D)),
            )
            load_x_chunk(nc.sync, 2)
            load_x_chunk(nc.sync, 3)

            # ---- projections ----
            def proj_qk(w_h, c, is_q):
                dst = qt_c[c] if is_q else kt_c[c]
                praws, sqs, nrms = [], [], []
                for hp in range(2):
                    pp = ps.tile([P, NQ], F32, tag="acc", bufs=2)
                    for i in range(ID):
                        nc.tensor.matmul(
                            pp[:],
                            lhsT=w_h[i // 4][:, i % 4, bass.ts(hp, P)],
                            rhs=xt_sb[i][c][:],
                            start=(i == 0),
                            stop=(i == ID - 1),
                        )
                    # fast PSUM evacuation: raw copy + square (both DVE)
                    praw = sb.tile([P, NQ], F16, tag="praw", bufs=3)
                    nc.vector.tensor_copy(praw[:], pp[:])
                    sq = sb.tile([P, NQ], F16, tag="sq", bufs=3)
                    nc.vector.tensor_mul(sq[:], praw[:], praw[:])
                    praws.append(praw)
                    sqs.append(sq)
                # su matmuls after both accumulations so the PE never
                # waits on the DVE praw/sq chain
                for hp in range(2):
                    su = ps.tile([P, NQ], F32, tag="acc", bufs=2)
                    nc.tensor.matmul(
                        su[0:2, :], lhsT=ones2_sb[:], rhs=sqs[hp][:],
                        start=True, stop=True,
                    )
                    # 1/sqrt(x) = exp(-0.5*ln(x)): stays in Exp/Ln table set
                    lnr = sb.tile([2, NQ], F32, tag="lnr", bufs=2)
                    nc.scalar.activation(lnr[:], su[0:2, :], AF.Ln)
                    nrm = sb.tile([2, NQ], F32R, tag="nrm", bufs=2)
                    nc.scalar.activation(nrm[:], lnr[:], AF.Exp, scale=-0.5)
                    nrms.append(nrm)
                # phase B after both ACT chains are in flight: the rb
                # matmuls land on the PE queue behind the second pp
                # accumulation, so the Ln/Exp latency is hidden
                for hp in range(2):
                    # partition-broadcast via PE: rows 0-63 <- nrm[0],
                    # rows 64-127 <- nrm[1]
                    rb = ps.tile([P, NQ], F32, tag="acc", bufs=2)
                    nc.tensor.matmul(
                        rb[:], lhsT=sel2_sb[:], rhs=nrms[hp][:],
                        start=True, stop=True,
                    )
                    nc.vector.tensor_mul(dst[:, hp, :], praws[hp][:], rb[:])

            def proj(c):
                proj_qk(wq_h, c, True)
                proj_qk(wk_h, c, False)
                # V: two 256-col accumulation groups share one bank
                for t2 in range(2):
                    pp = ps.tile([P, NQ], F32, tag="acc", bufs=2)
                    for half in range(2):
                        tt = 2 * t2 + half
                        for i in range(ID):
                            nc.tensor.matmul(
                                pp[:, bass.ts(half, J)],
                                lhsT=xt_sb[i][c][:, bass.ts(tt, P)],
                                rhs=wv_h[i // 4][:, i % 4, :],
                                start=(i == 0),
                                stop=(i == ID - 1),
                            )
                    nc.vector.tensor_copy(
                        v_c[c][:, 2 * t2 : 2 * t2 + 2, :, 0:DH],
                        pp[:].rearrange("p (t h x) -> p t h x", t=2, x=DH),
                    )

            # ---- attention ----
            vhat = {}
            CH = 2

            def attn(qc):
                nkt = 4 * qc + 4
                for hp in range(2):
                    ots = [
                        ps.tile([P, NQ], F32, tag="ot", bufs=2, name=f"ot{i}")
                        for i in range(2)
                    ]
                    for c0 in range(0, nkt, CH):
                        kts = range(c0, min(c0 + CH, nkt))
                        sts = {}
                        for kt in kts:
                            dj = kt - 4 * qc
                            q0 = P * dj if dj >= 1 else 0
                            st2 = ps.tile([P, 2, NQ], F32, tag="st2", bufs=2)
                            for h01 in range(2):
                                h64 = slice(64 * h01, 64 * h01 + 64)
                                nc.tensor.matmul(
                                    st2[:, h01, q0:],
                                    lhsT=kt_c[kt // 4][
                                        h64, hp, bass.ts(kt % 4, P)
                                    ],
                                    rhs=qt_c[qc][h64, hp, q0:],
                                    start=True,
                                    stop=True,
                                )
                            sts[kt] = st2
                        for kt in kts:
                            dj = kt - 4 * qc
                            q0 = P * dj if dj >= 1 else 0
                            # pt2[:, :, 0:q0] is never read (the AV matmul
                            # is range-restricted), so no zeroing
                            pt2 = sb.tile([P, 2, NQ], F16, tag="pt2", bufs=4)
                            nc.scalar.activation(
                                pt2[:, :, q0:],
                                sts[kt][:, :, q0:],
                                AF.Exp,
                                scale=1.0 / math.sqrt(DH),
                            )
                            if dj >= 0:
                                blk = slice(P * dj, P * dj + P)
                                nc.vector.tensor_mul(
                                    pt2[:, :, blk], pt2[:, :, blk], mask2_sb[:]
                                )
                            for h01 in range(2):
                                nc.tensor.matmul(
                                    ots[h01][0:VW, q0:],
                                    lhsT=v_c[kt // 4][:, kt % 4, 2 * hp + h01, 0:VW],
                                    rhs=pt2[:, h01, q0:],
                                    start=(kt == 0),
                                    stop=(kt == nkt - 1),
                                    skip_group_check=True,
                                )
                    # evacuate the AV accumulators to SBUF right away
                    # (frees both PSUM banks for the next hp pipeline),
                    # then the whole denominator chain runs on DVE: a
                    # native reciprocal of the two den rows into the
                    # zeroed staging tile, one f32r selector matmul to
                    # broadcast (rows 0-63 <- den66[1], 64-127 <- [65]).
                    oc = sb.tile(
                        [VW, 2, NQ], F32R, tag=f"oc{hp}", name=f"oc{hp}_{qc}"
                    )
                    nc.vector.tensor_copy(oc[:, 0, :], ots[0][0:VW, :])
                    nc.vector.tensor_copy(oc[:, 1, :], ots[1][0:VW, :])
                    rbo = ps.tile([P, NQ], F32, tag="acc", bufs=2)
                    nc.tensor.matmul(
                        rbo[:], lhsT=selp_sb[:, 0, :], rhs=oc[:, 0, :],
                        start=True, stop=False,
                    )
                    nc.tensor.matmul(
                        rbo[:], lhsT=selp_sb[:, 1, :], rhs=oc[:, 1, :],
                        start=False, stop=True,
                    )
                    # 1/x = exp(-ln(x)) on ACT (same table set as the
                    # attention exps; DVE reciprocal is ~6.5ns/elem)
                    lnd = sb.tile([P, NQ], F32, tag="lnd", bufs=2)
                    nc.scalar.activation(lnd[:], rbo[:], AF.Ln)
                    # rbos stays in PSUM: the vh muls pair it with the
                    # SBUF oc at a different base partition, which the
                    # verifier only allows for mixed SB/PSUM inputs
                    rbos = ps.tile([P, NQ], F32, tag="acc", bufs=2)
                    nc.scalar.activation(rbos[:], lnd[:], AF.Exp, scale=-1.0)
                    vh = sb.tile(
                        [P, NQ], F16, tag=f"vh{hp}_{qc}", name=f"vh{hp}_{qc}"
                    )
                    nc.vector.tensor_mul(
                        vh[0:64, :], oc[0:DH, 0, :], rbos[0:64, :]
                    )
                    nc.vector.tensor_mul(
                        vh[64:128, :], oc[0:DH, 1, :], rbos[64:128, :]
                    )
                    vhat[(hp, qc)] = vh

            # ---- (lagged) output projection + split RS ----
            rs_ccs = []

            def oproj(qc):
                for mc in range(2):
                    msl = bass.ts(mc, NQ)
                    ysb4 = sb.tile([P, 4, NQ], F16, tag="ysb4", bufs=2)
                    for t4 in range(4):
                        yp = ps.tile([P, NQ], F32, tag="acc", bufs=2)
                        for hp in range(2):
                            nc.tensor.matmul(
                                yp[:],
                                lhsT=vhat[(hp, qc)][:, bass.ts(t4, P)],
                                rhs=wo_sb[:, hp, msl],
                                start=(hp == 0),
                                stop=(hp == 1),
                            )
                        nc.vector.tensor_add(
                            ysb4[:, t4, :], yp[:], bias_sb[:, msl]
                        )
                    if qc < NQC - 1:
                        nc.sync.dma_start(
                            yparts[qc][mc].ap().rearrange("t p q -> p t q"),
                            ysb4[:],
                        )
                        cc = nc.gpsimd.collective_compute(
                            "ReduceScatter",
                            mybir.AluOpType.add,
                            replica_groups=[[0, 1, 2, 3], [4, 5, 6, 7]],
                            ins=[yparts[qc][mc].ap()],
                            outs=[yrss[qc][mc].ap()],
                        )
                        rs_ccs.append((qc, mc, cc))
                    else:
                        nc.sync.dma_start(
                            ypart3.ap()[:, :, mc * NQ : mc * NQ + NQ]
                            .rearrange("t p q -> p t q"),
                            ysb4[:],
                        )
                if qc == NQC - 1:
                    cc = nc.gpsimd.collective_compute(
                        "ReduceScatter",
                        mybir.AluOpType.add,
                        replica_groups=[[0, 1, 2, 3], [4, 5, 6, 7]],
                        ins=[ypart3.ap()],
                        outs=[yrs3.ap()],
                    )
                    rs_ccs.append((qc, None, cc))

            # interleaved emission: attention on chunk qc runs while later
            # chunks' projections are in flight. The big chunk (3) runs
            # mid-kernel; its ReduceScatter overlaps attn(1)/attn(2).
            proj(0)
            proj(1)
            attn(0)
            oproj(0)
            proj(2)
            attn(1)
            oproj(1)
            proj(3)
            attn(2)
            oproj(2)
            attn(3)
            oproj(3)

            # RS -> y_ext copies: emitted last on the scalar queue. A
            # DMA that waits on its collective stalls the whole issuing
            # engine, so these must sit behind ALL other scalar work
            # (and off the sync queue, which carries the ypart stores).
            for qc, mc, cc in rs_ccs:
                if mc is None:
                    outdma = nc.scalar.dma_start(
                        y_ext[qc].rearrange("m p q -> p m q"),
                        yrs3.ap().rearrange("p (m q) -> p m q", m=2),
                    )
                else:
                    outdma = nc.scalar.dma_start(
                        y_ext[qc, mc], yrss[qc][mc].ap()
                    )
                add_dep(outdma.ins, cc.ins, sync=True, reason="out after rs")

    _split_excess_waits(nc)
    return nc


_NC = None


def _get_nc():
    global _NC
    if _NC is None:
        _NC = _build()
    return _NC


def _make_in_maps(X, Wq, Wk, Wv, Wo, bo):
    BF = np.float16
    X = np.asarray(X, dtype=np.float32)
    Wq = np.asarray(Wq, dtype=np.float32)
    Wk = np.asarray(Wk, dtype=np.float32)
    Wv = np.asarray(Wv, dtype=np.float32)
    Wo = np.asarray(Wo, dtype=np.float32)
    bo = np.asarray(bo, dtype=np.float32)

    r = np.arange(P)
    mask = (r[:, None] <= r[None, :]).astype(BF)
    mask2 = np.stack([mask, mask], axis=1)  # [P, 2, P]
    ones2 = np.zeros((P, 2), dtype=BF)
    ones2[0:64, 0] = 1
    ones2[64:128, 1] = 1
    sel2 = np.zeros((2, P), dtype=np.float32)
    sel2[0, 0:64] = 1
    sel2[1, 64:128] = 1
    selp = np.zeros((VW, 2, P), dtype=np.float32)
    selp[64, 0, 0:64] = 1
    selp[64, 1, 64:128] = 1
    bias4 = (bo * 0.25).astype(np.float32)
    # pre-tiled XT: [i, c, 128, 512] contiguous blocks of X[b].T
    xts = [
        np.ascontiguousarray(
            X[b].T.reshape(ID, P, NQC, NQ).transpose(0, 2, 1, 3)
        ).astype(BF)
        for b in range(B)
    ]

    def wslice(W, jsl):
        # [1024, 256] -> [2, 128, 4, 256] half-major contiguous blocks
        return np.ascontiguousarray(
            W[:, jsl].reshape(2, 4, P, J).transpose(0, 2, 1, 3)
        ).astype(BF)

    in_maps = []
    for c in range(8):
        b, g = c // 4, c % 4
        jsl = slice(g * J, (g + 1) * J)
        in_maps.append(
            {
                "xt": xts[b],
                "wq": wslice(Wq, jsl),
                "wk": wslice(Wk, jsl),
                "wv": wslice(Wv, jsl),
                "wo": np.ascontiguousarray(
                    Wo[jsl, :].reshape(2, P, D).transpose(1, 0, 2)
                ).astype(BF),
                "bias4": bias4,
                "mask2d": mask2,
                "ones2d": ones2,
                "sel2d": sel2,
                "selpd": selp,
            }
        )
    return in_maps


def _gather(res):
    out = np.empty((B, N, D), np.float32)
    for c in range(8):
        b, r = c // 4, c % 4
        yc = np.asarray(res.results[c]["y"], dtype=np.float32)
        for qc in range(NQC):
            rows = slice(NQ * qc + P * r, NQ * qc + P * r + P)
            out[b, rows, 0:NQ] = yc[qc, 0]
            out[b, rows, NQ:D] = yc[qc, 1]
    return out


def kernel(X, Wq, Wk, Wv, Wo, bo):
    nc = _get_nc()
    in_maps = _make_in_maps(X, Wq, Wk, Wv, Wo, bo)
    res = run_bass_kernel_spmd(nc, in_maps, list(range(8)))
    return _gather(res)


# revision 19
# speedup vs baseline: 1.0134x; 1.0134x over previous
"""Causal attention with L2-normalized Q/K — Trainium2 Bass kernel.

Problem shapes (hardcoded): X [2, 2048, 1024], Wq/Wk/Wv [1024, 1024],
Wo [1024, 1024], bo [1024]; H=16 heads, d_head=64.

Sharding: 8 cores = 2 batches x 4 head-groups (4 heads each).
Core c handles batch b=c//4, heads 4*(c%4)..4*(c%4)+3.
Each core computes QKV projections for its head slice, per-head
normalized causal attention, and a partial output projection
V_hat @ Wo[slice]. The partials are summed with per-q-chunk bf16
ReduceScatters across the 4 cores of the batch; the host reassembles
the row strips.

v3 design notes (vs the v2 baseline at ~277-310us; this version ~255us):
- Q is stored in the SAME natural pair layout as K (head 2hp at
  partitions 0:64, head 2hp+1 at 64:128). Score matmuls contract over
  64 partitions with matching base offsets (PE cost is per moving
  column, independent of contraction depth), so no zero-padding DMAs
  and half-size score LDWEIGHTS. This also fixes v2's head pairing:
  v2 overwrote its 2 Q slots per head-pair so all scores used the
  last pair's Q (numerically masked by the near-uniform attention of
  L2-normalized Q/K, but ~9e-3 of avoidable error).
- Score pairs land in one 2-bank PSUM tile [128, 2, 512] so ONE ACT
  exp covers both heads of a pair per k-tile (ACT per-instruction
  overhead ~300ns; halving instruction count saves ~45us of ACT).
  Diagonal mask multiply also covers the pair in one DVE op.
- Softmax denominators: ots are evacuated to SBUF immediately after
  the AV accumulation stops (frees PSUM banks fast), the two den rows
  get a native-DVE reciprocal (no ACT Ln/Exp chain, no table traffic),
  and one f32r selector matmul broadcasts both reciprocal rows across
  partitions.
- Input loads are split per (i-tile, chunk) and spread across the two
  HWDGE queues (sync + scalar): proj(0) needs only wq + chunk-0 xt
  (1.5MB) instead of the whole 5.5MB preload, moving first-matmul
  from ~18us to ~6us.
- PSUM: score-pair pool 2x[128,2,512] (4 banks) + 2 AV accumulators
  + 2 general banks (projection/su/rb/rbo/yp) = 8 banks exactly.
- Output: fp16 ReduceScatter of the [4,128,512] output-projection
  partials, two per chunk for chunks 0-2 (halves overlap compute) and
  one merged 1MB RS for the last chunk (RS cost is latency-dominated,
  so one window beats two serial ones on the exposed tail). The
  RS->y_ext copies are emitted last on the scalar queue: a DMA that
  waits on its collective stalls the whole issuing engine.
"""

import math
import numpy as np
from contextlib import ExitStack

import concourse.bass as bass
import concourse.tile as tile
from concourse import mybir
from concourse.bass import _add_dep_helper as add_dep
from concourse.bass_utils import run_bass_kernel_spmd

F32 = mybir.dt.float32
F32R = mybir.dt.float32r
F16 = mybir.dt.float16
AF = mybir.ActivationFunctionType

B, N, D, H, DH = 2, 2048, 1024, 16, 64
NH = 4            # heads per core
J = NH * DH       # head dims per core = 256
P = 128
NQ = 512          # q chunk (moving free dim / psum bank)
NKT = N // P      # 16 k-tiles per head
ID = D // P       # 8 i-tiles of d_model
VW = DH + 1       # 65: V columns + ones column
NQC = N // NQ     # 4 q-chunks

_MAX_WAITS = 1


def _split_excess_waits(nc, limit=_MAX_WAITS):
    """This walrus build allows very few sem waits per instruction.
    Tile can emit many (kernel-tail Drain, collectives reading
    many-writer DRAM). Move excess waits onto injected same-engine
    NoOps right before the instruction; in-order execution preserves
    the semantics."""
    ctr = 0
    for fn in nc.m.functions:
        for bb in fn.blocks:
            out = []
            changed = False
            for ins in bb.instructions:
                si = ins.sync_info
                waits = list(si.on_wait) if si and si.on_wait else []
                if len(waits) > limit:
                    changed = True
                    chunks = [
                        waits[i : i + limit] for i in range(0, len(waits), limit)
                    ]
                    for ch in chunks[:-1]:
                        nop = mybir.InstNoOp(
                            name=f"I-waitsplit-{ctr}", ins=[], outs=[]
                        )
                        ctr += 1
                        nop.engine = ins.engine
                        nop.sync_info = mybir.SyncInfo(on_wait=ch, on_update=[])
                        out.append(nop)
                    ins.sync_info = mybir.SyncInfo(
                        on_wait=chunks[-1], on_update=list(si.on_update or [])
                    )
                out.append(ins)
            if changed:
                bb.instructions = out


def _build():
    nc = bass.Bass("TRN2", target_bir_lowering=False, debug=False, num_devices=8)

    xt = nc.dram_tensor("xt", [ID, NQC, P, NQ], F16, kind="ExternalInput").ap()
    wq = nc.dram_tensor("wq", [2, P, 4, J], F16, kind="ExternalInput").ap()
    wk = nc.dram_tensor("wk", [2, P, 4, J], F16, kind="ExternalInput").ap()
    wv = nc.dram_tensor("wv", [2, P, 4, J], F16, kind="ExternalInput").ap()
    wo = nc.dram_tensor("wo", [P, 2, D], F16, kind="ExternalInput").ap()
    bias4 = nc.dram_tensor("bias4", [D], F32, kind="ExternalInput").ap()
    mask2d = nc.dram_tensor("mask2d", [P, 2, P], F16, kind="ExternalInput").ap()
    ones2d = nc.dram_tensor("ones2d", [P, 2], F16, kind="ExternalInput").ap()
    sel2d = nc.dram_tensor("sel2d", [2, P], F32R, kind="ExternalInput").ap()
    selpd = nc.dram_tensor("selpd", [VW, 2, P], F32R, kind="ExternalInput").ap()
    # output: per q-chunk, 2 column halves of this core's 128-row strip
    y_ext = nc.dram_tensor(
        "y", [NQC, 2, P, NQ], F16, kind="ExternalOutput"
    ).ap()

    yparts = [
        [nc.dram_tensor(f"ypart{qc}_{mc}", [4, P, NQ], F16) for mc in range(2)]
        for qc in range(NQC - 1)
    ]
    yrss = [
        [nc.dram_tensor(f"yrs{qc}_{mc}", [P, NQ], F16) for mc in range(2)]
        for qc in range(NQC - 1)
    ]
    # last chunk: one 1MB RS (a single ~20us window beats two serial
    # ~13us ones on the exposed tail)
    ypart3 = nc.dram_tensor("ypart3", [4, P, D], F16)
    yrs3 = nc.dram_tensor("yrs3", [P, D], F16)

    with tile.TileContext(nc) as tc:
        with ExitStack() as ctx:
            sb = ctx.enter_context(tc.tile_pool(name="sb", bufs=1))
            ps = ctx.enter_context(tc.tile_pool(name="ps", bufs=1, space="PSUM"))

            # ---- loads: split per (i, c) and spread across the two
            # HWDGE queues (sync + scalar). gpsimd SW-DGE loads hang
            # the device; vector/tensor cannot issue DMA on TRN2. ----
            def load_w_half(eng, ap_in, nm, h):
                t = sb.tile([P, 4, J], F16, tag=f"{nm}{h}", name=f"{nm}{h}")
                eng.dma_start(t[:], ap_in[h])
                return t

            # critical path: wq + half of chunk-0 x on sync, the other
            # half of chunk-0 x then wk/wv on scalar
            wq_h = [load_w_half(nc.sync, wq, "wq", h) for h in range(2)]

            xt_sb = [[None] * NQC for _ in range(ID)]

            def load_x(eng, i, c):
                t = sb.tile([P, NQ], F16, tag=f"x{i}_{c}", name=f"x{i}_{c}")
                eng.dma_start(t[:], xt[i, c])
                xt_sb[i][c] = t

            def load_x_chunk(eng, c):
                for i in range(ID):
                    load_x(eng, i, c)

            for i in range(4):
                load_x(nc.scalar, i, 0)
            for i in range(4, ID):
                load_x(nc.sync, i, 0)
            wk_h = [load_w_half(nc.scalar, wk, "wk", h) for h in range(2)]
            wv_h = [load_w_half(nc.scalar, wv, "wv", h) for h in range(2)]
            load_x_chunk(nc.sync, 1)

            # small constants on sync, ordered by first use (scalar
            # engine carries only wk/wv: its DMA instructions execute
            # ahead of all its ACT work, so anything queued there
            # delays every norm chain)
            mask2_sb = sb.tile([P, 2, P], F16, tag="mask2")
            nc.sync.dma_start(mask2_sb[:], mask2d)
            ones2_sb = sb.tile([P, 2], F16, tag="ones2")
            nc.sync.dma_start(ones2_sb[:], ones2d)
            sel2_sb = sb.tile([2, P], F32R, tag="sel2")
            nc.sync.dma_start(sel2_sb[:], sel2d)
            selp_sb = sb.tile([VW, 2, P], F32R, tag="selp")
            nc.sync.dma_start(selp_sb[:], selpd)

            # ---- static SBUF state (per q-chunk tiles; Q uses the same
            # natural pair layout as K: head 2hp at partitions 0:64,
            # head 2hp+1 at 64:128 of slot hp) ----
            qt_c = [
                sb.tile([P, 2, NQ], F16, tag=f"qtc{c}", name=f"qtc{c}")
                for c in range(NQC)
            ]
            kt_c = [
                sb.tile([P, 2, NQ], F16, tag=f"ktc{c}", name=f"ktc{c}")
                for c in range(NQC)
            ]
            v_c = [
                sb.tile([P, 4, NH, VW], F16, tag=f"vc{c}", name=f"vc{c}")
                for c in range(NQC)
            ]
            # set the V ones column via gpsimd memset (a broadcast DMA
            # here costs ~20us: 2048 two-byte descriptors)
            for c in range(NQC):
                nc.gpsimd.memset(v_c[c][:, :, :, DH : DH + 1], 1.0)
            wo_sb = sb.tile([P, 2, D], F16, tag="wo")
            nc.sync.dma_start(wo_sb[:], wo)
            bias_sb = sb.tile([P, D], F32, tag="bias")
            nc.sync.dma_start(
                bias_sb[:],
                bias4.rearrange("(a m) -> a m", a=1).to_broadcast((P, D)),
            )
            load_x_chunk(nc.sync, 2)
            load_x_chunk(nc.sync, 3)

            # ---- projections ----
            def proj_qk(w_h, c, is_q):
                dst = qt_c[c] if is_q else kt_c[c]
                praws, sqs, nrms = [], [], []
                for hp in range(2):
                    pp = ps.tile([P, NQ], F32, tag="acc", bufs=2)
                    for i in range(ID):
                        nc.tensor.matmul(
                            pp[:],
                            lhsT=w_h[i // 4][:, i % 4, bass.ts(hp, P)],
                            rhs=xt_sb[i][c][:],
                            start=(i == 0),
                            stop=(i == ID - 1),
                        )
                    # fast PSUM evacuation: raw copy + square (both DVE)
                    praw = sb.tile([P, NQ], F16, tag="praw", bufs=3)
                    nc.vector.tensor_copy(praw[:], pp[:])
                    sq = sb.tile([P, NQ], F16, tag="sq", bufs=3)
                    nc.vector.tensor_mul(sq[:], praw[:], praw[:])
                    praws.append(praw)
                    sqs.append(sq)
                # su matmuls after both accumulations so the PE never
                # waits on the DVE praw/sq chain
                for hp in range(2):
                    su = ps.tile([P, NQ], F32, tag="acc", bufs=2)
                    nc.tensor.matmul(
                        su[0:2, :], lhsT=ones2_sb[:], rhs=sqs[hp][:],
                        start=True, stop=True,
                    )
                    # 1/sqrt(x) = exp(-0.5*ln(x)): stays in Exp/Ln table set
                    lnr = sb.tile([2, NQ], F32, tag="lnr", bufs=2)
                    nc.scalar.activation(lnr[:], su[0:2, :], AF.Ln)
                    nrm = sb.tile([2, NQ], F32R, tag="nrm", bufs=2)
                    nc.scalar.activation(nrm[:], lnr[:], AF.Exp, scale=-0.5)
                    nrms.append(nrm)
                # phase B after both ACT chains are in flight: the rb
                # matmuls land on the PE queue behind the second pp
                # accumulation, so the Ln/Exp latency is hidden
                for hp in range(2):
                    # partition-broadcast via PE: rows 0-63 <- nrm[0],
                    # rows 64-127 <- nrm[1]
                    rb = ps.tile([P, NQ], F32, tag="acc", bufs=2)
                    nc.tensor.matmul(
                        rb[:], lhsT=sel2_sb[:], rhs=nrms[hp][:],
                        start=True, stop=True,
                    )
                    nc.vector.tensor_mul(dst[:, hp, :], praws[hp][:], rb[:])

            def proj(c):
                proj_qk(wq_h, c, True)
                proj_qk(wk_h, c, False)
                # V: two 256-col accumulation groups share one bank
                for t2 in range(2):
                    pp = ps.tile([P, NQ], F32, tag="acc", bufs=2)
                    for half in range(2):
                        tt = 2 * t2 + half
                        for i in range(ID):
                            nc.tensor.matmul(
                                pp[:, bass.ts(half, J)],
                                lhsT=xt_sb[i][c][:, bass.ts(tt, P)],
                                rhs=wv_h[i // 4][:, i % 4, :],
                                start=(i == 0),
                                stop=(i == ID - 1),
                            )
                    nc.vector.tensor_copy(
                        v_c[c][:, 2 * t2 : 2 * t2 + 2, :, 0:DH],
                        pp[:].rearrange("p (t h x) -> p t h x", t=2, x=DH),
                    )

            # ---- attention ----
            vhat = {}
            CH = 2

            def attn(qc):
                nkt = 4 * qc + 4
                for hp in range(2):
                    ots = [
                        ps.tile([P, NQ], F32, tag="ot", bufs=2, name=f"ot{i}")
                        for i in range(2)
                    ]
                    for c0 in range(0, nkt, CH):
                        kts = range(c0, min(c0 + CH, nkt))
                        sts = {}
                        for kt in kts:
                            dj = kt - 4 * qc
                            q0 = P * dj if dj >= 1 else 0
                            st2 = ps.tile([P, 2, NQ], F32, tag="st2", bufs=2)
                            for h01 in range(2):
                                h64 = slice(64 * h01, 64 * h01 + 64)
                                nc.tensor.matmul(
                                    st2[:, h01, q0:],
                                    lhsT=kt_c[kt // 4][
                                        h64, hp, bass.ts(kt % 4, P)
                                    ],
                                    rhs=qt_c[qc][h64, hp, q0:],
                                    start=True,
                                    stop=True,
                                )
                            sts[kt] = st2
                        for kt in kts:
                            dj = kt - 4 * qc
                            q0 = P * dj if dj >= 1 else 0
                            # pt2[:, :, 0:q0] is never read (the AV matmul
                            # is range-restricted), so no zeroing
                            pt2 = sb.tile([P, 2, NQ], F16, tag="pt2", bufs=4)
                            nc.scalar.activation(
                                pt2[:, :, q0:],
                                sts[kt][:, :, q0:],
                                AF.Exp,
                                scale=1.0 / math.sqrt(DH),
                            )
                            if dj >= 0:
                                blk = slice(P * dj, P * dj + P)
                                nc.vector.tensor_mul(
                                    pt2[:, :, blk], pt2[:, :, blk], mask2_sb[:]
                                )
                            for h01 in range(2):
                                nc.tensor.matmul(
                                    ots[h01][0:VW, q0:],
                                    lhsT=v_c[kt // 4][:, kt % 4, 2 * hp + h01, 0:VW],
                                    rhs=pt2[:, h01, q0:],
                                    start=(kt == 0),
                                    stop=(kt == nkt - 1),
                                    skip_group_check=True,
                                )
                    # evacuate the AV accumulators to SBUF right away
                    # (frees both PSUM banks for the next hp pipeline),
                    # then the whole denominator chain runs on DVE: a
                    # native reciprocal of the two den rows into the
                    # zeroed staging tile, one f32r selector matmul to
                    # broadcast (rows 0-63 <- den66[1], 64-127 <- [65]).
                    oc = sb.tile(
                        [VW, 2, NQ], F32R, tag=f"oc{hp}", name=f"oc{hp}_{qc}"
                    )
                    nc.vector.tensor_copy(oc[:, 0, :], ots[0][0:VW, :])
                    nc.vector.tensor_copy(oc[:, 1, :], ots[1][0:VW, :])
                    rbo = ps.tile([P, NQ], F32, tag="acc", bufs=2)
                    nc.tensor.matmul(
                        rbo[:], lhsT=selp_sb[:, 0, :], rhs=oc[:, 0, :],
                        start=True, stop=False,
                    )
                    nc.tensor.matmul(
                        rbo[:], lhsT=selp_sb[:, 1, :], rhs=oc[:, 1, :],
                        start=False, stop=True,
                    )
                    # 1/x = exp(-ln(x)) on ACT (same table set as the
                    # attention exps; DVE reciprocal is ~6.5ns/elem)
                    lnd = sb.tile([P, NQ], F32, tag="lnd", bufs=2)
                    nc.scalar.activation(lnd[:], rbo[:], AF.Ln)
                    # rbos stays in PSUM: the vh muls pair it with the
                    # SBUF oc at a different base partition, which the
                    # verifier only allows for mixed SB/PSUM inputs
                    rbos = ps.tile([P, NQ], F32, tag="acc", bufs=2)
                    nc.scalar.activation(rbos[:], lnd[:], AF.Exp, scale=-1.0)
                    vh = sb.tile(
                        [P, NQ], F16, tag=f"vh{hp}_{qc}", name=f"vh{hp}_{qc}"
                    )
                    nc.vector.tensor_mul(
                        vh[0:64, :], oc[0:DH, 0, :], rbos[0:64, :]
                    )
                    nc.vector.tensor_mul(
                        vh[64:128, :], oc[0:DH, 1, :], rbos[64:128, :]
                    )
                    vhat[(hp, qc)] = vh

            # ---- (lagged) output projection + split RS ----
            rs_ccs = []

            def oproj(qc):
                for mc in range(2):
                    msl = bass.ts(mc, NQ)
                    ysb4 = sb.tile([P, 4, NQ], F16, tag="ysb4", bufs=2)
                    for t4 in range(4):
                        yp = ps.tile([P, NQ], F32, tag="acc", bufs=2)
                        for hp in range(2):
                            nc.tensor.matmul(
                                yp[:],
                                lhsT=vhat[(hp, qc)][:, bass.ts(t4, P)],
                                rhs=wo_sb[:, hp, msl],
                                start=(hp == 0),
                                stop=(hp == 1),
                            )
                        nc.vector.tensor_add(
                            ysb4[:, t4, :], yp[:], bias_sb[:, msl]
                        )
                    if qc < NQC - 1:
                        nc.sync.dma_start(
                            yparts[qc][mc].ap().rearrange("t p q -> p t q"),
                            ysb4[:],
                        )
                        cc = nc.gpsimd.collective_compute(
                            "ReduceScatter",
                            mybir.AluOpType.add,
                            replica_groups=[[0, 1, 2, 3], [4, 5, 6, 7]],
                            ins=[yparts[qc][mc].ap()],
                            outs=[yrss[qc][mc].ap()],
                        )
                        rs_ccs.append((qc, mc, cc))
                    else:
                        nc.sync.dma_start(
                            ypart3.ap()[:, :, mc * NQ : mc * NQ + NQ]
                            .rearrange("t p q -> p t q"),
                            ysb4[:],
                        )
                if qc == NQC - 1:
                    cc = nc.gpsimd.collective_compute(
                        "ReduceScatter",
                        mybir.AluOpType.add,
                        replica_groups=[[0, 1, 2, 3], [4, 5, 6, 7]],
                        ins=[ypart3.ap()],
                        outs=[yrs3.ap()],
                    )
                    rs_ccs.append((qc, None, cc))

            # interleaved emission: attention on chunk qc runs while later
            # chunks' projections are in flight. The big chunk (3) runs
            # mid-kernel; its ReduceScatter overlaps attn(1)/attn(2).
            proj(0)
            proj(1)
            attn(0)
            oproj(0)
            proj(2)
            attn(1)
            oproj(1)
            proj(3)
            attn(2)
            oproj(2)
            attn(3)
            oproj(3)

            # RS -> y_ext copies: emitted last on the scalar queue. A
            # DMA that waits on its collective stalls the whole issuing
            # engine, so these must sit behind ALL other scalar work
            # (and off the sync queue, which carries the ypart stores).
            for qc, mc, cc in rs_ccs:
                if mc is None:
                    outdma = nc.scalar.dma_start(
                        y_ext[qc].rearrange("m p q -> p m q"),
                        yrs3.ap().rearrange("p (m q) -> p m q", m=2),
                    )
                else:
                    outdma = nc.scalar.dma_start(
                        y_ext[qc, mc], yrss[qc][mc].ap()
                    )
                add_dep(outdma.ins, cc.ins, sync=True, reason="out after rs")

    _split_excess_waits(nc)
    return nc


_NC = None


def _get_nc():
    global _NC
    if _NC is None:
        _NC = _build()
    return _NC


def _make_in_maps(X, Wq, Wk, Wv, Wo, bo):
    BF = np.float16
    X = np.asarray(X, dtype=np.float32)
    Wq = np.asarray(Wq, dtype=np.float32)
    Wk = np.asarray(Wk, dtype=np.float32)
    Wv = np.asarray(Wv, dtype=np.float32)
    Wo = np.asarray(Wo, dtype=np.float32)
    bo = np.asarray(bo, dtype=np.float32)

    r = np.arange(P)
    mask = (r[:, None] <= r[None, :]).astype(BF)
    mask2 = np.stack([mask, mask], axis=1)  # [P, 2, P]
    ones2 = np.zeros((P, 2), dtype=BF)
    ones2[0:64, 0] = 1
    ones2[64:128, 1] = 1
    sel2 = np.zeros((2, P), dtype=np.float32)
    sel2[0, 0:64] = 1
    sel2[1, 64:128] = 1
    selp = np.zeros((VW, 2, P), dtype=np.float32)
    selp[64, 0, 0:64] = 1
    selp[64, 1, 64:128] = 1
    bias4 = (bo * 0.25).astype(np.float32)
    # pre-tiled XT: [i, c, 128, 512] contiguous blocks of X[b].T
    xts = [
        np.ascontiguousarray(
            X[b].T.reshape(ID, P, NQC, NQ).transpose(0, 2, 1, 3)
        ).astype(BF)
        for b in range(B)
    ]

    def wslice(W, jsl):
        # [1024, 256] -> [2, 128, 4, 256] half-major contiguous blocks
        return np.ascontiguousarray(
            W[:, jsl].reshape(2, 4, P, J).transpose(0, 2, 1, 3)
        ).astype(BF)

    in_maps = []
    for c in range(8):
        b, g = c // 4, c % 4
        jsl = slice(g * J, (g + 1) * J)
        in_maps.append(
            {
                "xt": xts[b],
                "wq": wslice(Wq, jsl),
                "wk": wslice(Wk, jsl),
                "wv": wslice(Wv, jsl),
                "wo": np.ascontiguousarray(
                    Wo[jsl, :].reshape(2, P, D).transpose(1, 0, 2)
                ).astype(BF),
                "bias4": bias4,
                "mask2d": mask2,
                "ones2d": ones2,
                "sel2d": sel2,
                "selpd": selp,
            }
        )
    return in_maps


def _gather(res):
    out = np.empty((B, N, D), np.float32)
    for c in range(8):
        b, r = c // 4, c % 4
        yc = np.asarray(res.results[c]["y"], dtype=np.float32)
        for qc in range(NQC):
            rows = slice(NQ * qc + P * r, NQ * qc + P * r + P)
            out[b, rows, 0:NQ] = yc[qc, 0]
            out[b, rows, NQ:D] = yc[qc, 1]
    return out


def kernel(X, Wq, Wk, Wv, Wo, bo):
    nc = _get_nc()
    in_maps = _make_in_maps(X, Wq, Wk, Wv, Wo, bo)
    res = run_bass_kernel_spmd(nc, in_maps, list(range(8)))
    return _gather(res)
